# revision 3
# baseline (speedup 1.0000x reference)
"""Trainium2 Bass kernel for nn_ConditionedMLP (FiLM-style conditioned MLP).

Math per layer:  x <- relu( einsum("bij,bj->bi", Y_b, x@W.T+b) + fb_b )
where  mod = side @ Fw.T + Fb,  Y_b = mod[:, :L*L].reshape(L, L),  fb_b = mod[:, -L:].

Key rewrite: with xh = x@W.T+b (per-sample) and kj a flattened (k, j) index,
  out[b, i] = sum_{k,j} Fw[i*L+j, k] * side[b,k] * xh[b,j]      (z-term)
            + sum_j    Fb[i*L+j] * xh[b,j]                       (Fb2-term)
            + sum_k    Fw[L*L+i, k] * side[b,k] + Fb[L*L+i]      (fb-term)
i.e. ONE matmul with contraction dim K*L(+L+K) over the outer-product tensor
z[(k,j), b] = side[b,k] * xh[b,j], which the tensor engine eats at full rate.

Everything runs in transposed layout (features on partitions, batch on the
free dim) so no on-chip transposes are needed anywhere: weights stream in as
pre-transposed bf16 super-chunks, z is built by DVE/GPSIMD tensor_tensor
against a resident broadcast-side tile, and each layer's relu output directly
feeds the next layer's matmul contraction.

Sharding: pure data parallel, batch 2048 -> 256 per core across 8 cores.
"""

import numpy as np
import ml_dtypes

import concourse.bass as bass
import concourse.mybir as mybir
import concourse.tile as tile
from concourse.vector_clock import ScopedClock
from concourse.bass_utils import run_bass_kernel_spmd

BF16 = ml_dtypes.bfloat16

B = 2048
NCORES = 8
BC = B // NCORES          # 256 batch per core
K = 64                    # side dim
L = 256                   # layer size
NIN = 128
NOUT = 64
NLAYERS = 3
NCHUNK = 2 * K            # 128 kj-chunks of 128 rows each (k-major, j inner)
SUPER = 16                # kj-chunks per DMA super-chunk (1 MB bf16)
NSUPER = NCHUNK // SUPER  # 8

_DT = mybir.dt


class _SplitDrainTileContext(tile.TileContext):
    """This container's walrus accepts only ONE sync-wait per instruction
    (setupSyncWait limit). Post-pass: hoist extra waits onto same-engine NOPs
    inserted immediately before the owning instruction — same semantics, the
    engine's FIFO enforces the conditions before the instruction issues."""

    def _drain_and_barrier(self, tick_clock, wait_clock):
        super()._drain_and_barrier(tick_clock, wait_clock)
        nc = self.nc
        cur_insts = nc.cur_bb.bb.instructions
        for bb in nc.main_func.blocks:
            insts = bb.instructions
            rewritten = []
            changed = False
            for inst in list(insts):
                si = inst.sync_info
                waits = list(si.on_wait) if si is not None and si.on_wait else []
                if len(waits) > 1:
                    changed = True
                    for w in waits[:-1]:
                        nop = nc.engines[inst.engine].nop(nofuse=True).ins
                        assert cur_insts[-1].name == nop.name
                        cur_insts.pop()
                        nop.sync_info = mybir.SyncInfo(on_wait=[w], on_update=[])
                        rewritten.append(nop)
                    si.on_wait = [waits[-1]]
                rewritten.append(inst)
            if changed:
                insts[:] = rewritten


def build_nc(gpsimd_frac=3):
    """Build the single-core program (SPMD across 8 cores).

    gpsimd_frac: every gpsimd_frac-th z-build tensor_tensor goes to GPSIMD
    (0 = all on DVE).
    """
    nc = bass.Bass()
    bf = _DT.bfloat16
    f32 = _DT.float32

    # ---- DRAM I/O (per-core shard layouts, host-prepped) ----
    d_xT = nc.declare_dram_parameter("xT", [NIN, BC], bf, isOutput=False)
    d_sideT = nc.declare_dram_parameter("sideT", [K, BC], bf, isOutput=False)
    d_sbc = nc.declare_dram_parameter("sbc", [128, K * BC], bf, isOutput=False)
    d_fhat, d_wT, d_fb2T, d_ffbT, d_brow, d_fbb = [], [], [], [], [], []
    for i in range(NLAYERS):
        nin = NIN if i == 0 else L
        mch = nin // 128
        d_fhat.append(
            nc.declare_dram_parameter(
                f"fhat{i}", [NSUPER, 128, SUPER * L], bf, isOutput=False
            )
        )
        d_wT.append(
            nc.declare_dram_parameter(f"wT{i}", [128, mch * L], bf, isOutput=False)
        )
        d_fb2T.append(
            nc.declare_dram_parameter(f"fb2T{i}", [128, 2 * L], bf, isOutput=False)
        )
        d_ffbT.append(
            nc.declare_dram_parameter(f"ffbT{i}", [K, L], bf, isOutput=False)
        )
        d_brow.append(
            nc.declare_dram_parameter(f"brow{i}", [1, L], bf, isOutput=False)
        )
        d_fbb.append(
            nc.declare_dram_parameter(f"fbb{i}", [128, 2], f32, isOutput=False)
        )
    d_woutT = nc.declare_dram_parameter("woutT", [128, 2 * NOUT], bf, isOutput=False)
    d_bout = nc.declare_dram_parameter("boutr", [1, NOUT], bf, isOutput=False)
    d_out = nc.declare_dram_parameter("o", [NOUT, BC], f32, isOutput=True)

    with _SplitDrainTileContext(nc) as tc:
        with (
            tc.tile_pool(name="consts", bufs=1) as consts,
            tc.tile_pool(name="wpool", bufs=3) as wpool,
            tc.tile_pool(name="lsm", bufs=2) as lsm,
            tc.tile_pool(name="xlin", bufs=2) as xlinp,
            tc.tile_pool(name="hp", bufs=2) as hp,
            tc.tile_pool(name="zp", bufs=8) as zp,
            tc.tile_pool(name="px", bufs=2, space="PSUM") as pxp,
            tc.tile_pool(name="po", bufs=2, space="PSUM") as pop,
            tc.tile_pool(name="pf", bufs=1, space="PSUM") as pfp,
            tc.tile_pool(name="outp", bufs=1) as outp,
        ):
            ones = consts.tile([1, BC], bf)
            nc.vector.memset(ones, 1.0)
            xT_sb = consts.tile([NIN, BC], bf)
            nc.sync.dma_start(out=xT_sb[:], in_=d_xT[:])
            sideT_sb = consts.tile([K, BC], bf)
            nc.sync.dma_start(out=sideT_sb[:], in_=d_sideT[:])
            sbc_sb = consts.tile([128, K * BC], bf)
            nc.sync.dma_start(out=sbc_sb[:], in_=d_sbc[:])

            h_sb = None  # previous layer output (transposed, bf16)
            tt_i = 0
            for li in range(NLAYERS):
                nin = NIN if li == 0 else L
                mch = nin // 128

                # --- per-layer weight loads ---
                w_sc = []
                for s in range(NSUPER):
                    t_w = wpool.tile([128, SUPER * L], bf, tag="fhat")
                    nc.sync.dma_start(out=t_w[:], in_=d_fhat[li][s])
                    w_sc.append(t_w)
                wT_sb = lsm.tile([128, mch * L], bf, tag="wT")
                nc.sync.dma_start(out=wT_sb[:], in_=d_wT[li][:])
                fb2T_sb = lsm.tile([128, 2 * L], bf, tag="fb2T")
                nc.sync.dma_start(out=fb2T_sb[:], in_=d_fb2T[li][:])
                ffbT_sb = lsm.tile([K, L], bf, tag="ffbT")
                nc.sync.dma_start(out=ffbT_sb[:], in_=d_ffbT[li][:])
                brow_sb = lsm.tile([1, L], bf, tag="brow")
                nc.sync.dma_start(out=brow_sb[:], in_=d_brow[li][:])
                fbb_sb = lsm.tile([128, 2], f32, tag="fbb")
                nc.sync.dma_start(out=fbb_sb[:], in_=d_fbb[li][:])

                # --- x_lin = W @ h + b  (transposed; i-chunks on partitions) ---
                rhs_prev = xT_sb if li == 0 else h_sb
                px = [pxp.tile([128, BC], f32, tag="px", name=f"px{ic}") for ic in range(2)]
                for ic in range(2):
                    for mc in range(mch):
                        nc.tensor.matmul(
                            px[ic][:],
                            lhsT=wT_sb[:, mc * L + ic * 128 : mc * L + ic * 128 + 128],
                            rhs=rhs_prev[:, mc * BC : (mc + 1) * BC]
                            if li > 0
                            else rhs_prev[:],
                            start=(mc == 0),
                            stop=False,
                        )
                    nc.tensor.matmul(
                        px[ic][:],
                        lhsT=brow_sb[:, ic * 128 : ic * 128 + 128],
                        rhs=ones[:],
                        start=False,
                        stop=True,
                    )
                xlin_sb = xlinp.tile([128, 2 * BC], bf, tag="xlin")
                for ic in range(2):
                    nc.scalar.activation(
                        out=xlin_sb[:, ic * BC : (ic + 1) * BC],
                        in_=px[ic][:],
                        func=mybir.ActivationFunctionType.Copy,
                    )

                # --- main accumulation: z chunks + Fb2 + fb terms ---
                po = [pop.tile([128, BC], f32, tag="po", name=f"po{ic}") for ic in range(2)]
                for kk in range(K):
                    z2 = zp.tile([128, 2 * BC], bf, tag="z")
                    # in1: side_bc k-block repeated across both j-halves
                    sb_blk = sbc_sb[:, kk * BC : (kk + 1) * BC]
                    sb_rep = bass.AP(
                        tensor=sb_blk.tensor,
                        offset=sb_blk.offset,
                        ap=[sb_blk.ap[0], [0, 2], sb_blk.ap[1]],
                    )
                    eng = (
                        nc.gpsimd
                        if (gpsimd_frac and tt_i % gpsimd_frac == gpsimd_frac - 1)
                        else nc.vector
                    )
                    tt_i += 1
                    eng.tensor_tensor(
                        out=z2[:].rearrange("p (r b) -> p r b", r=2),
                        in0=xlin_sb[:].rearrange("p (r b) -> p r b", r=2),
                        in1=sb_rep,
                        op=mybir.AluOpType.mult,
                    )
                    for jh in range(2):
                        c = kk * 2 + jh
                        s, t = c // SUPER, c % SUPER
                        for ic in range(2):
                            nc.tensor.matmul(
                                po[ic][:],
                                lhsT=w_sc[s][
                                    :, t * L + ic * 128 : t * L + ic * 128 + 128
                                ],
                                rhs=z2[:, jh * BC : (jh + 1) * BC],
                                start=(c == 0),
                                stop=False,
                            )
                for jh in range(2):
                    for ic in range(2):
                        nc.tensor.matmul(
                            po[ic][:],
                            lhsT=fb2T_sb[:, jh * L + ic * 128 : jh * L + ic * 128 + 128],
                            rhs=xlin_sb[:, jh * BC : (jh + 1) * BC],
                            start=False,
                            stop=False,
                        )
                for ic in range(2):
                    nc.tensor.matmul(
                        po[ic][:],
                        lhsT=ffbT_sb[:, ic * 128 : ic * 128 + 128],
                        rhs=sideT_sb[:],
                        start=False,
                        stop=True,
                    )

                # --- relu + fb bias; becomes next layer's (transposed) input ---
                h_sb = hp.tile([128, 2 * BC], bf, tag="h")
                for ic in range(2):
                    nc.scalar.activation(
                        out=h_sb[:, ic * BC : (ic + 1) * BC],
                        in_=po[ic][:],
                        func=mybir.ActivationFunctionType.Relu,
                        bias=fbb_sb[:, ic : ic + 1],
                    )

            # --- final projection out = h @ Wout.T + bout (transposed) ---
            woutT_sb = consts.tile([128, 2 * NOUT], bf)
            nc.sync.dma_start(out=woutT_sb[:], in_=d_woutT[:])
            bout_sb = consts.tile([1, NOUT], bf)
            nc.sync.dma_start(out=bout_sb[:], in_=d_bout[:])
            pf = pfp.tile([NOUT, BC], f32)
            for mc in range(2):
                nc.tensor.matmul(
                    pf[:],
                    lhsT=woutT_sb[:, mc * NOUT : (mc + 1) * NOUT],
                    rhs=h_sb[:, mc * BC : (mc + 1) * BC],
                    start=(mc == 0),
                    stop=False,
                )
            nc.tensor.matmul(
                pf[:], lhsT=bout_sb[:], rhs=ones[:], start=False, stop=True
            )
            out_sb = outp.tile([NOUT, BC], f32)
            nc.vector.tensor_copy(out_sb[:], pf[:])
            nc.sync.dma_start(out=d_out[:], in_=out_sb[:])

    return nc


def _prep_inputs(inputs):
    """Host-side slicing + layout transforms. Returns per-core input maps."""

    def b16(a):
        return np.ascontiguousarray(a.astype(BF16))

    side = inputs["side"].astype(np.float32)
    x = inputs["x"].astype(np.float32)

    xT = b16(x.T)                      # [NIN, B]
    sideT = b16(side.T)                # [K, B]

    shared = {}
    for i in range(NLAYERS):
        Fw = inputs[f"W{i}"], inputs[f"b{i}"]
        W, bvec = Fw
        Fwf, Fbf = inputs[f"Fw{i}"].astype(np.float32), inputs[f"Fb{i}"].astype(
            np.float32
        )
        nin = NIN if i == 0 else L
        mch = nin // 128
        # FhatT[(k*L + j), ii] = Fw[ii*L + j, k]; packed into DMA super-chunks
        fhatT = Fwf[: L * L].reshape(L, L, K).transpose(2, 1, 0).reshape(K * L, L)
        fhat_sc = (
            fhatT.reshape(NSUPER, SUPER, 128, L)
            .transpose(0, 2, 1, 3)
            .reshape(NSUPER, 128, SUPER * L)
        )
        shared[f"fhat{i}"] = b16(fhat_sc)
        # W.T packed by m-chunk: wT_sc[p, mc*L+ii] = W.T[mc*128+p, ii]
        wT = W.astype(np.float32).T  # [nin, L]
        shared[f"wT{i}"] = b16(
            wT.reshape(mch, 128, L).transpose(1, 0, 2).reshape(128, mch * L)
        )
        # Fb2T[j, ii] = Fb[ii*L + j], packed by j-chunk
        fb2T = Fbf[: L * L].reshape(L, L).T  # [j, ii]
        shared[f"fb2T{i}"] = b16(
            fb2T.reshape(2, 128, L).transpose(1, 0, 2).reshape(128, 2 * L)
        )
        shared[f"ffbT{i}"] = b16(Fwf[L * L :].T)       # [K, L]
        shared[f"brow{i}"] = b16(bvec.astype(np.float32)[None, :])  # [1, L]
        shared[f"fbb{i}"] = np.ascontiguousarray(
            Fbf[L * L :].reshape(2, 128).T.astype(np.float32)
        )  # [128, 2]
    woutT = inputs["Wout"].astype(np.float32).T  # [L, NOUT]
    shared["woutT"] = b16(
        woutT.reshape(2, 128, NOUT).transpose(1, 0, 2).reshape(128, 2 * NOUT)
    )
    shared["boutr"] = b16(inputs["bout"].astype(np.float32)[None, :])

    in_maps = []
    for c in range(NCORES):
        sl = slice(c * BC, (c + 1) * BC)
        m = dict(shared)
        m["xT"] = np.ascontiguousarray(xT[:, sl])
        m["sideT"] = np.ascontiguousarray(sideT[:, sl])
        # broadcast side: sbc[p, k*BC + b] = side[b0+b, k] for every p
        flat = np.ascontiguousarray(sideT[:, sl]).reshape(1, K * BC)
        m["sbc"] = np.ascontiguousarray(np.broadcast_to(flat, (128, K * BC)))
        in_maps.append(m)
    return in_maps


_NC_CACHE = {}


def _get_nc(gpsimd_frac=3):
    if gpsimd_frac not in _NC_CACHE:
        _NC_CACHE[gpsimd_frac] = build_nc(gpsimd_frac)
    return _NC_CACHE[gpsimd_frac]


def run(inputs, trace=False, gpsimd_frac=3):
    nc = _get_nc(gpsimd_frac)
    in_maps = _prep_inputs(inputs)
    res = run_bass_kernel_spmd(nc, in_maps, list(range(NCORES)), trace=trace)
    out = np.concatenate([res.results[c]["o"] for c in range(NCORES)], axis=1)
    return np.ascontiguousarray(out.T.astype(np.float32)), res


def kernel(**inputs):
    return run(inputs, trace=False)[0]
